# revision 11
# baseline (speedup 1.0000x reference)
"""Causal self-attention on 8 trn2 NeuronCores.

Sharding: core c -> (batch b = c//2, head-group g = c%2).  Each head-group
is 8 heads = 512 channels.  Per core:
  - q/k/v projections of x[b] restricted to the group's 512 columns
  - causal attention for the 8 heads, computed in the transposed
    orientation S^T = [tk, tq] so softmax denominators come from a
    ones-column appended to V (AV matmul yields them for free) and no
    transposes are needed anywhere
  - partial output projection through the group's 512 rows of Wo
Host sums the two partials per batch and adds (bv @ Wo + bo): softmax
weights sum to one, so the v-bias passes through attention additively.

QK uses PE row tiling: each head's K=64 score matmul loads its k tile
at row offset 0 or 64 of the PE array (tile_position auto-derived from
the operands' base partition), so the two heads of a 128-channel group
execute concurrently on the two halves of the array — 2x QK throughput
vs the zero-padded full-footprint formulation, and both row halves stay
active so the PE activity monitor keeps the clock at 2.4 GHz.

Scheduling: projections, attention and the output projection are
software-pipelined per 512-column window.  The attention pair loop for
window w pulls "filler" work — the projections for window w+1 and the
output projection for window w-1 — between pairs, so the tensor engine
chews independent matmuls while the scalar engine (exp, the critical
path) catches up.  This starts exp ~80us earlier than a phase-separated
emission and backfills the PE stall before each AV matmul.
"""

import numpy as np
import ml_dtypes

import concourse.bass as bass
import concourse.mybir as mybir
from concourse import bacc, tile
from concourse.bass_utils import run_bass_kernel_spmd

B, T, C, H = 4, 2048, 1024, 16
HD = C // H          # 64
G = 2                # head groups (cores per batch)
HG = H // G          # 8 heads per group
CG = C // G          # 512 channels per group
CGP = CG // 128      # 4 c_out tiles per group
P = 128
W = 512              # free-dim window (one PSUM bank of f32)
NW = T // W          # 4 windows
NTT = T // P         # 16 t tiles
NCI = C // P         # 8 c_in chunks
VS = HD + 1          # 65: v plus ones column

_cached_nc = None


def _build():
    f32 = mybir.dt.float32
    bf16 = mybir.dt.bfloat16
    AF = mybir.ActivationFunctionType
    nc = bacc.Bacc("TRN2", target_bir_lowering=False, debug=False, num_devices=8)

    xt_d = nc.dram_tensor("xt", [C, T], bf16, kind="ExternalInput")
    wq_d = nc.dram_tensor("wq", [C, CG], bf16, kind="ExternalInput")
    wk_d = nc.dram_tensor("wk", [C, CG], bf16, kind="ExternalInput")
    wv_d = nc.dram_tensor("wv", [C, CG], bf16, kind="ExternalInput")
    wo_d = nc.dram_tensor("wo", [CG, C], bf16, kind="ExternalInput")
    bq_d = nc.dram_tensor("bq", [P, CGP], f32, kind="ExternalInput")
    bk_d = nc.dram_tensor("bk", [P, CGP], f32, kind="ExternalInput")
    mask_d = nc.dram_tensor("mask", [P, 2 * P], bf16, kind="ExternalInput")
    out_d = nc.dram_tensor("outp", [C, T], bf16, kind="ExternalOutput")

    mm = lambda out, lhsT, rhs, start, stop: nc.tensor.matmul(
        out, lhsT, rhs, start=start, stop=stop)

    with tile.TileContext(nc) as tc:
        with (
            tc.tile_pool(name="pers", bufs=1) as pers,
            tc.tile_pool(name="xchunk", bufs=NCI) as xpool,
            tc.tile_pool(name="wchunk", bufs=1) as wpool,
            # shared by q/k/v projection accumulators and the out-proj
            tc.tile_pool(name="psum_b", bufs=2, space="PSUM") as psum_b,
            tc.tile_pool(name="attn", bufs=1) as attn,
            tc.tile_pool(name="psum_mm", bufs=2, space="PSUM") as psum_mm,
            tc.tile_pool(name="pt", bufs=8) as ptpool,
            tc.tile_pool(name="dn", bufs=4) as dnpool,
            tc.tile_pool(name="rb", bufs=4) as rbpool,
            tc.tile_pool(name="psum_av", bufs=2, space="PSUM") as psum_av,
            tc.tile_pool(name="osb", bufs=3) as opool,
        ):
            qT = pers.tile([P, CGP, T], bf16)        # q^T: [c_out, t]
            kT = pers.tile([P, CGP, T], bf16)        # k^T: [c_out, t]
            vp = pers.tile([P, NTT, HG * VS], bf16)  # v rows + ones col/head
            wo_sb = pers.tile([P, CGP, C], bf16)
            # triangular mask duplicated on a middle dim so one DVE mul
            # covers both heads of a group; bf16 keeps the fast DVE modes
            maskD = pers.tile([P, 2, P], bf16)
            bq_sb = pers.tile([P, CGP], f32)
            bk_sb = pers.tile([P, CGP], f32)
            ones_sb = pers.tile([P, HG], f32)
            yT = attn.tile([P, CGP, T], bf16)

            nc.vector.memset(ones_sb, 1.0)

            wh = wpool.tile([P, 3, NCI, CG], bf16)
            # DMA order matters for the first matmul: wv chunks 0-3, then
            # the x first-window columns, then the rest.
            nc.sync.dma_start(
                out=wh[:, 2, 0:4, :],
                in_=wv_d.ap().rearrange("(c p) n -> p c n", p=P)[:, 0:4, :])
            xc = []
            for ci in range(NCI):
                t_ = xpool.tile([P, T], bf16, tag="xc")
                nc.sync.dma_start(out=t_[:, 0:W],
                                  in_=xt_d.ap()[ci * P:(ci + 1) * P, 0:W])
                xc.append(t_)
            nc.sync.dma_start(
                out=wh[:, 2, 4:NCI, :],
                in_=wv_d.ap().rearrange("(c p) n -> p c n", p=P)[:, 4:NCI, :])
            nc.sync.dma_start(
                out=wh[:, 0, :, :],
                in_=wq_d.ap().rearrange("(c p) n -> p c n", p=P))
            nc.sync.dma_start(
                out=wh[:, 1, :, :],
                in_=wk_d.ap().rearrange("(c p) n -> p c n", p=P))
            for ci in range(NCI):
                nc.sync.dma_start(out=xc[ci][:, W:],
                                  in_=xt_d.ap()[ci * P:(ci + 1) * P, W:])
            nc.sync.dma_start(out=bq_sb, in_=bq_d.ap())
            nc.sync.dma_start(out=bk_sb, in_=bk_d.ap())
            nc.sync.dma_start(out=maskD.rearrange("p a b -> p (a b)"),
                              in_=mask_d.ap())
            nc.sync.dma_start(
                out=wo_sb, in_=wo_d.ap().rearrange("(c p) n -> p c n", p=P))

            def emit_q(j, w):
                ws = slice(w * W, (w + 1) * W)
                psq = psum_b.tile([P, W], f32, tag="psb")
                for i in range(NCI):
                    mm(psq, wh[:, 0, i, j * P:(j + 1) * P],
                       xc[i][:, ws], start=(i == 0), stop=(i == NCI - 1))
                nc.vector.tensor_scalar_add(qT[:, j, ws], psq,
                                            bq_sb[:, j:j + 1])

            def emit_k(j, w):
                ws = slice(w * W, (w + 1) * W)
                psk = psum_b.tile([P, W], f32, tag="psb")
                for i in range(NCI):
                    mm(psk, wh[:, 1, i, j * P:(j + 1) * P],
                       xc[i][:, ws], start=(i == 0), stop=(i == NCI - 1))
                nc.vector.tensor_scalar_add(kT[:, j, ws], psk,
                                            bk_sb[:, j:j + 1])

            def emit_v(it):
                psv = psum_b.tile([P, W], f32, tag="psb")
                for i in range(NCI):
                    mm(psv, xc[i][:, it * P:(it + 1) * P], wh[:, 2, i, :],
                       start=(i == 0), stop=(i == NCI - 1))
                v_view = vp[:, it, :].rearrange(
                    "p (h x) -> p h x", x=VS)[:, :, 0:HD]
                nc.vector.tensor_copy(
                    v_view, psv.rearrange("p (h x) -> p h x", x=HD))
                ones_view = vp[:, it, :].rearrange(
                    "p (h x) -> p h x", x=VS)[:, :, HD:VS]
                nc.vector.tensor_copy(
                    ones_view, ones_sb.rearrange("p (h x) -> p h x", x=1))

            def emit_o(m, w):
                tq0 = w * W
                po = psum_b.tile([P, W], f32, tag="psb")
                for i in range(CGP):
                    mm(po, wo_sb[:, i, m * P:(m + 1) * P],
                       yT[:, i, tq0:tq0 + W],
                       start=(i == 0), stop=(i == CGP - 1))
                ot = opool.tile([P, W], bf16, tag="ot")
                nc.vector.tensor_copy(ot, po)
                nc.sync.dma_start(
                    out=out_d.ap()[m * P:(m + 1) * P, tq0:tq0 + W],
                    in_=ot)

            def proj_items(w):
                items = [(lambda it=it: emit_v(it))
                         for it in range(4 * w, 4 * w + 4)]
                for j in range(CGP):
                    items.append(lambda j=j, w=w: emit_q(j, w))
                    items.append(lambda j=j, w=w: emit_k(j, w))
                return items

            escale = 1.0 / float(np.sqrt(HD))

            # window 0 projections up front; everything after is pipelined
            for fn in proj_items(0):
                fn()

            for w in range(NW):
                tq0 = w * W
                ntk = (w + 1) * (W // P)
                filler = []
                if w + 1 < NW:
                    filler += proj_items(w + 1)
                if w >= 1:
                    filler += [(lambda m=m, w=w: emit_o(m, w - 1))
                               for m in range(C // P)]
                nsteps = CGP * ntk
                fidx = 0
                acc = 0.0
                per_step = len(filler) / nsteps
                for j in range(CGP):
                    # both heads of the group accumulate concurrently
                    ps_av0 = psum_av.tile([VS, W], f32, tag="av")
                    ps_av1 = psum_av.tile([VS, W], f32, tag="av")
                    ps_av = [ps_av0, ps_av1]
                    for i in range(ntk):
                        vs0 = max(tq0, i * P)
                        n0 = tq0 + W - vs0
                        ps_s = psum_mm.tile([P, 2, W], f32, tag="ps")
                        pt = ptpool.tile([P, 2, W], bf16, tag="pt")
                        ts = slice(i * P, (i + 1) * P)
                        # K=64 per head at PE row offsets 0/64: the two
                        # matmuls run on separate array halves concurrently
                        mm(ps_s[:, 0, W - n0:], kT[0:HD, j, ts],
                           qT[0:HD, j, vs0:vs0 + n0], start=True, stop=True)
                        mm(ps_s[:, 1, W - n0:], kT[HD:P, j, ts],
                           qT[HD:P, j, vs0:vs0 + n0], start=True, stop=True)
                        nc.scalar.activation(pt[:, :, W - n0:],
                                             ps_s[:, :, W - n0:],
                                             AF.Exp, scale=escale)
                        if i * P >= tq0:  # diagonal tile: one dual-head mul
                            nc.vector.tensor_mul(
                                pt[:, :, W - n0:W - n0 + P],
                                pt[:, :, W - n0:W - n0 + P], maskD)
                        for a in range(2):
                            h = 2 * j + a
                            mm(ps_av[a][:, vs0 - tq0:],
                               vp[:, i, h * VS:(h + 1) * VS],
                               pt[:, a, W - n0:],
                               start=(i == 0), stop=(i == ntk - 1))
                        acc += per_step
                        while acc >= 1.0 and fidx < len(filler):
                            filler[fidx]()
                            fidx += 1
                            acc -= 1.0
                    # normalize: reciprocal on the 1-row denominator, then
                    # broadcast the reciprocal (cheaper than broadcasting
                    # and dividing on [64, W])
                    for a in range(2):
                        dn = dnpool.tile([1, W], f32, tag="dn")
                        nc.vector.tensor_copy(dn, ps_av[a][HD:VS, :])
                        nc.vector.reciprocal_approx_fast(out=dn, in_=dn)
                        rb = rbpool.tile([HD, W], f32, tag="rb")
                        nc.gpsimd.partition_broadcast(rb, dn)
                        nc.vector.tensor_mul(
                            yT[a * HD:(a + 1) * HD, j, tq0:tq0 + W],
                            ps_av[a][0:HD, :], rb)
                while fidx < len(filler):
                    filler[fidx]()
                    fidx += 1
            for m in range(C // P):
                emit_o(m, NW - 1)

    nc.compile()
    return nc


def get_nc():
    global _cached_nc
    if _cached_nc is None:
        _cached_nc = _build()
    return _cached_nc


def make_in_maps(x, Wq, bq, Wk, bk, Wv, bv, Wo, bo):
    x = np.asarray(x, np.float32)
    tri = np.triu(np.ones((P, P), np.float32))
    mask = np.concatenate([tri, tri], axis=1).astype(ml_dtypes.bfloat16)
    in_maps = []
    for c in range(8):
        b, g = c // 2, c % 2
        cs = slice(g * CG, (g + 1) * CG)
        in_maps.append({
            "xt": np.ascontiguousarray(x[b].T.astype(ml_dtypes.bfloat16)),
            "wq": np.ascontiguousarray(
                np.asarray(Wq, np.float32)[:, cs].astype(ml_dtypes.bfloat16)),
            "wk": np.ascontiguousarray(
                np.asarray(Wk, np.float32)[:, cs].astype(ml_dtypes.bfloat16)),
            "wv": np.ascontiguousarray(
                np.asarray(Wv, np.float32)[:, cs].astype(ml_dtypes.bfloat16)),
            "wo": np.ascontiguousarray(
                np.asarray(Wo, np.float32)[cs, :].astype(ml_dtypes.bfloat16)),
            "bq": np.ascontiguousarray(
                np.asarray(bq, np.float32)[cs].reshape(CGP, P).T),
            "bk": np.ascontiguousarray(
                np.asarray(bk, np.float32)[cs].reshape(CGP, P).T),
            "mask": np.ascontiguousarray(mask),
        })
    return in_maps


def combine(results, Wv, bv, Wo, bo):
    const = (np.asarray(bv, np.float32) @ np.asarray(Wo, np.float32)
             + np.asarray(bo, np.float32))
    out = np.empty((B, T, C), np.float32)
    for b in range(B):
        acc = (results[2 * b]["outp"].astype(np.float32)
               + results[2 * b + 1]["outp"].astype(np.float32))
        out[b] = acc.T + const[None, :]
    return out


def kernel(x, Wq, bq, Wk, bk, Wv, bv, Wo, bo):
    nc = get_nc()
    in_maps = make_in_maps(x, Wq, bq, Wk, bk, Wv, bv, Wo, bo)
    res = run_bass_kernel_spmd(nc, in_maps, core_ids=list(range(8)))
    return combine(res.results, Wv, bv, Wo, bo)


# revision 13
# speedup vs baseline: 1.0769x; 1.0769x over previous
"""Causal self-attention on 8 trn2 NeuronCores.

Sharding: core c -> (batch b = c//2, head-group g = c%2).  Each head-group
is 8 heads = 512 channels.  Per core:
  - q/k/v projections of x[b] restricted to the group's 512 columns
  - causal attention for the 8 heads, computed in the transposed
    orientation S^T = [tk, tq] so softmax denominators come from a
    ones-column appended to V (AV matmul yields them for free) and no
    transposes are needed anywhere
  - partial output projection through the group's 512 rows of Wo
Host sums the two partials per batch and adds (bv @ Wo + bo): softmax
weights sum to one, so the v-bias passes through attention additively.

QK uses PE row tiling: each head's K=64 score matmul loads its k tile
at row offset 0 or 64 of the PE array (tile_position auto-derived from
the operands' base partition), so the two heads of a 128-channel group
execute concurrently on the two halves of the array — 2x QK throughput
vs the zero-padded full-footprint formulation, and both row halves stay
active so the PE activity monitor keeps the clock at 2.4 GHz.

Scheduling: projections, attention and the output projection are
software-pipelined per 512-column window.  The attention pair loop for
window w pulls "filler" work — the projections for window w+1 and the
output projection for window w-1 — between pairs, so the tensor engine
chews independent matmuls while the scalar engine (exp, the critical
path) catches up.  This starts exp ~80us earlier than a phase-separated
emission and backfills the PE stall before each AV matmul.
"""

import numpy as np
import ml_dtypes

import concourse.bass as bass
import concourse.mybir as mybir
from concourse import bacc, tile
from concourse.bass_utils import run_bass_kernel_spmd

B, T, C, H = 4, 2048, 1024, 16
HD = C // H          # 64
G = 2                # head groups (cores per batch)
HG = H // G          # 8 heads per group
CG = C // G          # 512 channels per group
CGP = CG // 128      # 4 c_out tiles per group
P = 128
W = 512              # free-dim window (one PSUM bank of f32)
NW = T // W          # 4 windows
NTT = T // P         # 16 t tiles
NCI = C // P         # 8 c_in chunks
VS = HD + 1          # 65: v plus ones column

_cached_nc = None


def _build():
    f32 = mybir.dt.float32
    bf16 = mybir.dt.bfloat16
    AF = mybir.ActivationFunctionType
    nc = bacc.Bacc("TRN2", target_bir_lowering=False, debug=False, num_devices=8)

    xt_d = nc.dram_tensor("xt", [C, T], bf16, kind="ExternalInput")
    wq_d = nc.dram_tensor("wq", [C, CG], bf16, kind="ExternalInput")
    wk_d = nc.dram_tensor("wk", [C, CG], bf16, kind="ExternalInput")
    wv_d = nc.dram_tensor("wv", [C, CG], bf16, kind="ExternalInput")
    wo_d = nc.dram_tensor("wo", [CG, C], bf16, kind="ExternalInput")
    bq_d = nc.dram_tensor("bq", [P, CGP], f32, kind="ExternalInput")
    bk_d = nc.dram_tensor("bk", [P, CGP], f32, kind="ExternalInput")
    mask_d = nc.dram_tensor("mask", [P, 2 * P], bf16, kind="ExternalInput")
    out_d = nc.dram_tensor("outp", [C, T], bf16, kind="ExternalOutput")

    mm = lambda out, lhsT, rhs, start, stop: nc.tensor.matmul(
        out, lhsT, rhs, start=start, stop=stop)

    with tile.TileContext(nc) as tc:
        with (
            tc.tile_pool(name="pers", bufs=1) as pers,
            tc.tile_pool(name="xchunk", bufs=NCI) as xpool,
            tc.tile_pool(name="wchunk", bufs=1) as wpool,
            # shared by q/k/v projection accumulators and the out-proj
            tc.tile_pool(name="psum_b", bufs=2, space="PSUM") as psum_b,
            tc.tile_pool(name="attn", bufs=1) as attn,
            tc.tile_pool(name="psum_mm", bufs=2, space="PSUM") as psum_mm,
            tc.tile_pool(name="pt", bufs=NTT + 1) as ptpool,
            tc.tile_pool(name="dn", bufs=4) as dnpool,
            tc.tile_pool(name="rb", bufs=4) as rbpool,
            tc.tile_pool(name="psum_av", bufs=2, space="PSUM") as psum_av,
            tc.tile_pool(name="osb", bufs=3) as opool,
        ):
            qT = pers.tile([P, CGP, T], bf16)        # q^T: [c_out, t]
            kT = pers.tile([P, CGP, T], bf16)        # k^T: [c_out, t]
            vp = pers.tile([P, NTT, HG * VS], bf16)  # v rows + ones col/head
            wo_sb = pers.tile([P, CGP, C], bf16)
            # triangular mask duplicated on a middle dim so one DVE mul
            # covers both heads of a group; bf16 keeps the fast DVE modes
            maskD = pers.tile([P, 2, P], bf16)
            bq_sb = pers.tile([P, CGP], f32)
            bk_sb = pers.tile([P, CGP], f32)
            ones_sb = pers.tile([P, HG], f32)
            yT = attn.tile([P, CGP, T], bf16)

            nc.vector.memset(ones_sb, 1.0)

            wh = wpool.tile([P, 3, NCI, CG], bf16)
            # DMA order matters for the first matmul: wv chunks 0-3, then
            # the x first-window columns, then the rest.
            nc.sync.dma_start(
                out=wh[:, 2, 0:4, :],
                in_=wv_d.ap().rearrange("(c p) n -> p c n", p=P)[:, 0:4, :])
            xc = []
            for ci in range(NCI):
                t_ = xpool.tile([P, T], bf16, tag="xc")
                nc.sync.dma_start(out=t_[:, 0:W],
                                  in_=xt_d.ap()[ci * P:(ci + 1) * P, 0:W])
                xc.append(t_)
            nc.sync.dma_start(
                out=wh[:, 2, 4:NCI, :],
                in_=wv_d.ap().rearrange("(c p) n -> p c n", p=P)[:, 4:NCI, :])
            nc.sync.dma_start(
                out=wh[:, 0, :, :],
                in_=wq_d.ap().rearrange("(c p) n -> p c n", p=P))
            nc.sync.dma_start(
                out=wh[:, 1, :, :],
                in_=wk_d.ap().rearrange("(c p) n -> p c n", p=P))
            for ci in range(NCI):
                nc.sync.dma_start(out=xc[ci][:, W:],
                                  in_=xt_d.ap()[ci * P:(ci + 1) * P, W:])
            nc.sync.dma_start(out=bq_sb, in_=bq_d.ap())
            nc.sync.dma_start(out=bk_sb, in_=bk_d.ap())
            nc.sync.dma_start(out=maskD.rearrange("p a b -> p (a b)"),
                              in_=mask_d.ap())
            nc.sync.dma_start(
                out=wo_sb, in_=wo_d.ap().rearrange("(c p) n -> p c n", p=P))

            def emit_q(j, w):
                ws = slice(w * W, (w + 1) * W)
                psq = psum_b.tile([P, W], f32, tag="psb")
                for i in range(NCI):
                    mm(psq, wh[:, 0, i, j * P:(j + 1) * P],
                       xc[i][:, ws], start=(i == 0), stop=(i == NCI - 1))
                nc.vector.tensor_scalar_add(qT[:, j, ws], psq,
                                            bq_sb[:, j:j + 1])

            def emit_k(j, w):
                ws = slice(w * W, (w + 1) * W)
                psk = psum_b.tile([P, W], f32, tag="psb")
                for i in range(NCI):
                    mm(psk, wh[:, 1, i, j * P:(j + 1) * P],
                       xc[i][:, ws], start=(i == 0), stop=(i == NCI - 1))
                nc.vector.tensor_scalar_add(kT[:, j, ws], psk,
                                            bk_sb[:, j:j + 1])

            def emit_v(it):
                psv = psum_b.tile([P, W], f32, tag="psb")
                for i in range(NCI):
                    mm(psv, xc[i][:, it * P:(it + 1) * P], wh[:, 2, i, :],
                       start=(i == 0), stop=(i == NCI - 1))
                v_view = vp[:, it, :].rearrange(
                    "p (h x) -> p h x", x=VS)[:, :, 0:HD]
                nc.vector.tensor_copy(
                    v_view, psv.rearrange("p (h x) -> p h x", x=HD))
                ones_view = vp[:, it, :].rearrange(
                    "p (h x) -> p h x", x=VS)[:, :, HD:VS]
                nc.vector.tensor_copy(
                    ones_view, ones_sb.rearrange("p (h x) -> p h x", x=1))

            def emit_o(m, w):
                tq0 = w * W
                po = psum_b.tile([P, W], f32, tag="psb")
                for i in range(CGP):
                    mm(po, wo_sb[:, i, m * P:(m + 1) * P],
                       yT[:, i, tq0:tq0 + W],
                       start=(i == 0), stop=(i == CGP - 1))
                ot = opool.tile([P, W], bf16, tag="ot")
                nc.vector.tensor_copy(ot, po)
                nc.sync.dma_start(
                    out=out_d.ap()[m * P:(m + 1) * P, tq0:tq0 + W],
                    in_=ot)

            def proj_items(w):
                items = [(lambda it=it: emit_v(it))
                         for it in range(4 * w, 4 * w + 4)]
                for j in range(CGP):
                    items.append(lambda j=j, w=w: emit_q(j, w))
                    items.append(lambda j=j, w=w: emit_k(j, w))
                return items

            escale = 1.0 / float(np.sqrt(HD))

            # window 0 projections up front; everything after is pipelined
            for fn in proj_items(0):
                fn()

            for w in range(NW):
                tq0 = w * W
                ntk = (w + 1) * (W // P)
                filler = []
                if w + 1 < NW:
                    filler += proj_items(w + 1)
                else:
                    # the last window has no projection filler; feed it all
                    # the deferred output projections instead
                    for wo_ in range(NW - 1):
                        filler += [(lambda m=m, w_=wo_: emit_o(m, w_))
                                   for m in range(C // P)]
                nsteps = CGP * ntk
                fidx = 0
                acc = 0.0
                per_step = len(filler) / nsteps
                for j in range(CGP):
                    # phase 1: scores + exp for all tk tiles (scalar-paced;
                    # filler keeps the PE busy), pt tiles persist
                    pts = []
                    for i in range(ntk):
                        vs0 = max(tq0, i * P)
                        n0 = tq0 + W - vs0
                        ps_s = psum_mm.tile([P, 2, W], f32, tag="ps")
                        pt = ptpool.tile([P, 2, W], bf16, tag="pt")
                        ts = slice(i * P, (i + 1) * P)
                        # K=64 per head at PE row offsets 0/64: the two
                        # matmuls run on separate array halves concurrently
                        mm(ps_s[:, 0, W - n0:], kT[0:HD, j, ts],
                           qT[0:HD, j, vs0:vs0 + n0], start=True, stop=True)
                        mm(ps_s[:, 1, W - n0:], kT[HD:P, j, ts],
                           qT[HD:P, j, vs0:vs0 + n0], start=True, stop=True)
                        nc.scalar.activation(pt[:, :, W - n0:],
                                             ps_s[:, :, W - n0:],
                                             AF.Exp, scale=escale)
                        if i * P >= tq0:  # diagonal tile: one dual-head mul
                            nc.vector.tensor_mul(
                                pt[:, :, W - n0:W - n0 + P],
                                pt[:, :, W - n0:W - n0 + P], maskD)
                        pts.append((pt, vs0, n0))
                        acc += per_step
                        while acc >= 1.0 and fidx < len(filler):
                            filler[fidx]()
                            fidx += 1
                            acc -= 1.0
                    # phase 2: per-head AV batches, double-buffered psum so
                    # head/group boundaries overlap with the norms
                    for a in range(2):
                        h = 2 * j + a
                        ps_av = psum_av.tile([VS, W], f32, tag="av")
                        for i in range(ntk):
                            pt, vs0, n0 = pts[i]
                            mm(ps_av[:, vs0 - tq0:],
                               vp[:, i, h * VS:(h + 1) * VS],
                               pt[:, a, W - n0:],
                               start=(i == 0), stop=(i == ntk - 1))
                        # normalize: reciprocal on the 1-row denominator,
                        # then broadcast the reciprocal
                        dn = dnpool.tile([1, W], f32, tag="dn")
                        nc.vector.tensor_copy(dn, ps_av[HD:VS, :])
                        nc.vector.reciprocal_approx_fast(out=dn, in_=dn)
                        rb = rbpool.tile([HD, W], f32, tag="rb")
                        nc.gpsimd.partition_broadcast(rb, dn)
                        nc.vector.tensor_mul(
                            yT[a * HD:(a + 1) * HD, j, tq0:tq0 + W],
                            ps_av[0:HD, :], rb)
                while fidx < len(filler):
                    filler[fidx]()
                    fidx += 1
            for m in range(C // P):
                emit_o(m, NW - 1)

    nc.compile()
    return nc


def get_nc():
    global _cached_nc
    if _cached_nc is None:
        _cached_nc = _build()
    return _cached_nc


def make_in_maps(x, Wq, bq, Wk, bk, Wv, bv, Wo, bo):
    x = np.asarray(x, np.float32)
    tri = np.triu(np.ones((P, P), np.float32))
    mask = np.concatenate([tri, tri], axis=1).astype(ml_dtypes.bfloat16)
    in_maps = []
    for c in range(8):
        b, g = c // 2, c % 2
        cs = slice(g * CG, (g + 1) * CG)
        in_maps.append({
            "xt": np.ascontiguousarray(x[b].T.astype(ml_dtypes.bfloat16)),
            "wq": np.ascontiguousarray(
                np.asarray(Wq, np.float32)[:, cs].astype(ml_dtypes.bfloat16)),
            "wk": np.ascontiguousarray(
                np.asarray(Wk, np.float32)[:, cs].astype(ml_dtypes.bfloat16)),
            "wv": np.ascontiguousarray(
                np.asarray(Wv, np.float32)[:, cs].astype(ml_dtypes.bfloat16)),
            "wo": np.ascontiguousarray(
                np.asarray(Wo, np.float32)[cs, :].astype(ml_dtypes.bfloat16)),
            "bq": np.ascontiguousarray(
                np.asarray(bq, np.float32)[cs].reshape(CGP, P).T),
            "bk": np.ascontiguousarray(
                np.asarray(bk, np.float32)[cs].reshape(CGP, P).T),
            "mask": np.ascontiguousarray(mask),
        })
    return in_maps


def combine(results, Wv, bv, Wo, bo):
    const = (np.asarray(bv, np.float32) @ np.asarray(Wo, np.float32)
             + np.asarray(bo, np.float32))
    out = np.empty((B, T, C), np.float32)
    for b in range(B):
        acc = (results[2 * b]["outp"].astype(np.float32)
               + results[2 * b + 1]["outp"].astype(np.float32))
        out[b] = acc.T + const[None, :]
    return out


def kernel(x, Wq, bq, Wk, bk, Wv, bv, Wo, bo):
    nc = get_nc()
    in_maps = make_in_maps(x, Wq, bq, Wk, bk, Wv, bv, Wo, bo)
    res = run_bass_kernel_spmd(nc, in_maps, core_ids=list(range(8)))
    return combine(res.results, Wv, bv, Wo, bo)


# revision 14
# speedup vs baseline: 1.0811x; 1.0039x over previous
"""Causal self-attention on 8 trn2 NeuronCores.

Sharding: core c -> (batch b = c//2, head-group g = c%2).  Each head-group
is 8 heads = 512 channels.  Per core:
  - q/k/v projections of x[b] restricted to the group's 512 columns
  - causal attention for the 8 heads, computed in the transposed
    orientation S^T = [tk, tq] so softmax denominators come from a
    ones-column appended to V (AV matmul yields them for free) and no
    transposes are needed anywhere
  - partial output projection through the group's 512 rows of Wo
Host sums the two partials per batch and adds (bv @ Wo + bo): softmax
weights sum to one, so the v-bias passes through attention additively.

QK uses PE row tiling: each head's K=64 score matmul loads its k tile
at row offset 0 or 64 of the PE array (tile_position auto-derived from
the operands' base partition), so the two heads of a 128-channel group
execute concurrently on the two halves of the array — 2x QK throughput
vs the zero-padded full-footprint formulation, and both row halves stay
active so the PE activity monitor keeps the clock at 2.4 GHz.

Scheduling: projections, attention and the output projection are
software-pipelined per 512-column window.  The attention pair loop for
window w pulls "filler" work — the projections for window w+1 and the
output projection for window w-1 — between pairs, so the tensor engine
chews independent matmuls while the scalar engine (exp, the critical
path) catches up.  This starts exp ~80us earlier than a phase-separated
emission and backfills the PE stall before each AV matmul.
"""

import numpy as np
import ml_dtypes

import concourse.bass as bass
import concourse.mybir as mybir
from concourse import bacc, tile
from concourse.bass_utils import run_bass_kernel_spmd

B, T, C, H = 4, 2048, 1024, 16
HD = C // H          # 64
G = 2                # head groups (cores per batch)
HG = H // G          # 8 heads per group
CG = C // G          # 512 channels per group
CGP = CG // 128      # 4 c_out tiles per group
P = 128
W = 512              # free-dim window (one PSUM bank of f32)
NW = T // W          # 4 windows
NTT = T // P         # 16 t tiles
NCI = C // P         # 8 c_in chunks
VS = HD + 1          # 65: v plus ones column

_cached_nc = None


def _build():
    f32 = mybir.dt.float32
    bf16 = mybir.dt.bfloat16
    AF = mybir.ActivationFunctionType
    nc = bacc.Bacc("TRN2", target_bir_lowering=False, debug=False, num_devices=8)

    xt_d = nc.dram_tensor("xt", [C, T], bf16, kind="ExternalInput")
    wq_d = nc.dram_tensor("wq", [C, CG], bf16, kind="ExternalInput")
    wk_d = nc.dram_tensor("wk", [C, CG], bf16, kind="ExternalInput")
    wv_d = nc.dram_tensor("wv", [C, CG], bf16, kind="ExternalInput")
    wo_d = nc.dram_tensor("wo", [CG, C], bf16, kind="ExternalInput")
    bq_d = nc.dram_tensor("bq", [P, CGP], f32, kind="ExternalInput")
    bk_d = nc.dram_tensor("bk", [P, CGP], f32, kind="ExternalInput")
    mask_d = nc.dram_tensor("mask", [P, 2 * P], bf16, kind="ExternalInput")
    out_d = nc.dram_tensor("outp", [C, T], bf16, kind="ExternalOutput")

    mm = lambda out, lhsT, rhs, start, stop: nc.tensor.matmul(
        out, lhsT, rhs, start=start, stop=stop)

    with tile.TileContext(nc) as tc:
        with (
            tc.tile_pool(name="pers", bufs=1) as pers,
            tc.tile_pool(name="xchunk", bufs=NCI) as xpool,
            tc.tile_pool(name="wchunk", bufs=1) as wpool,
            # shared by q/k/v projection accumulators and the out-proj
            tc.tile_pool(name="psum_b", bufs=2, space="PSUM") as psum_b,
            tc.tile_pool(name="attn", bufs=1) as attn,
            tc.tile_pool(name="psum_mm", bufs=2, space="PSUM") as psum_mm,
            tc.tile_pool(name="pt", bufs=NTT + 1) as ptpool,
            tc.tile_pool(name="dn", bufs=4) as dnpool,
            tc.tile_pool(name="rb", bufs=4) as rbpool,
            tc.tile_pool(name="psum_av", bufs=2, space="PSUM") as psum_av,
            tc.tile_pool(name="osb", bufs=3) as opool,
        ):
            qT = pers.tile([P, CGP, T], bf16)        # q^T: [c_out, t]
            kT = pers.tile([P, CGP, T], bf16)        # k^T: [c_out, t]
            vp = pers.tile([P, NTT, HG * VS], bf16)  # v rows + ones col/head
            wo_sb = pers.tile([P, CGP, C], bf16)
            # triangular mask duplicated on a middle dim so one DVE mul
            # covers both heads of a group; bf16 keeps the fast DVE modes
            maskD = pers.tile([P, 2, P], bf16)
            bq_sb = pers.tile([P, CGP], f32)
            bk_sb = pers.tile([P, CGP], f32)
            ones_sb = pers.tile([P, HG], f32)
            yT = attn.tile([P, CGP, T], bf16)

            nc.vector.memset(ones_sb, 1.0)

            wh = wpool.tile([P, 3, NCI, CG], bf16)
            # DMA order matters for the first matmul: wv chunks 0-3, then
            # the x first-window columns, then the rest.
            nc.sync.dma_start(
                out=wh[:, 2, 0:4, :],
                in_=wv_d.ap().rearrange("(c p) n -> p c n", p=P)[:, 0:4, :])
            xc = []
            for ci in range(NCI):
                t_ = xpool.tile([P, T], bf16, tag="xc")
                nc.sync.dma_start(out=t_[:, 0:W],
                                  in_=xt_d.ap()[ci * P:(ci + 1) * P, 0:W])
                xc.append(t_)
            nc.sync.dma_start(
                out=wh[:, 2, 4:NCI, :],
                in_=wv_d.ap().rearrange("(c p) n -> p c n", p=P)[:, 4:NCI, :])
            nc.sync.dma_start(
                out=wh[:, 0, :, :],
                in_=wq_d.ap().rearrange("(c p) n -> p c n", p=P))
            nc.sync.dma_start(
                out=wh[:, 1, :, :],
                in_=wk_d.ap().rearrange("(c p) n -> p c n", p=P))
            # stagger the rest of x per window so the projection filler for
            # window w+1 never waits on a monolithic second-half transfer
            for ci in range(NCI):
                nc.sync.dma_start(out=xc[ci][:, W:2 * W],
                                  in_=xt_d.ap()[ci * P:(ci + 1) * P, W:2 * W])
            for ci in range(NCI):
                nc.sync.dma_start(out=xc[ci][:, 2 * W:],
                                  in_=xt_d.ap()[ci * P:(ci + 1) * P, 2 * W:])
            nc.sync.dma_start(out=bq_sb, in_=bq_d.ap())
            nc.sync.dma_start(out=bk_sb, in_=bk_d.ap())
            nc.sync.dma_start(out=maskD.rearrange("p a b -> p (a b)"),
                              in_=mask_d.ap())
            nc.sync.dma_start(
                out=wo_sb, in_=wo_d.ap().rearrange("(c p) n -> p c n", p=P))

            def emit_q(j, w):
                ws = slice(w * W, (w + 1) * W)
                psq = psum_b.tile([P, W], f32, tag="psb")
                for i in range(NCI):
                    mm(psq, wh[:, 0, i, j * P:(j + 1) * P],
                       xc[i][:, ws], start=(i == 0), stop=(i == NCI - 1))
                nc.vector.tensor_scalar_add(qT[:, j, ws], psq,
                                            bq_sb[:, j:j + 1])

            def emit_k(j, w):
                ws = slice(w * W, (w + 1) * W)
                psk = psum_b.tile([P, W], f32, tag="psb")
                for i in range(NCI):
                    mm(psk, wh[:, 1, i, j * P:(j + 1) * P],
                       xc[i][:, ws], start=(i == 0), stop=(i == NCI - 1))
                nc.vector.tensor_scalar_add(kT[:, j, ws], psk,
                                            bk_sb[:, j:j + 1])

            def emit_v(it):
                psv = psum_b.tile([P, W], f32, tag="psb")
                for i in range(NCI):
                    mm(psv, xc[i][:, it * P:(it + 1) * P], wh[:, 2, i, :],
                       start=(i == 0), stop=(i == NCI - 1))
                v_view = vp[:, it, :].rearrange(
                    "p (h x) -> p h x", x=VS)[:, :, 0:HD]
                nc.vector.tensor_copy(
                    v_view, psv.rearrange("p (h x) -> p h x", x=HD))
                ones_view = vp[:, it, :].rearrange(
                    "p (h x) -> p h x", x=VS)[:, :, HD:VS]
                nc.vector.tensor_copy(
                    ones_view, ones_sb.rearrange("p (h x) -> p h x", x=1))

            def emit_o(m, w):
                tq0 = w * W
                po = psum_b.tile([P, W], f32, tag="psb")
                for i in range(CGP):
                    mm(po, wo_sb[:, i, m * P:(m + 1) * P],
                       yT[:, i, tq0:tq0 + W],
                       start=(i == 0), stop=(i == CGP - 1))
                ot = opool.tile([P, W], bf16, tag="ot")
                nc.vector.tensor_copy(ot, po)
                nc.sync.dma_start(
                    out=out_d.ap()[m * P:(m + 1) * P, tq0:tq0 + W],
                    in_=ot)

            def proj_items(w):
                items = [(lambda it=it: emit_v(it))
                         for it in range(4 * w, 4 * w + 4)]
                for j in range(CGP):
                    items.append(lambda j=j, w=w: emit_q(j, w))
                    items.append(lambda j=j, w=w: emit_k(j, w))
                return items

            escale = 1.0 / float(np.sqrt(HD))

            # window 0 projections up front; everything after is pipelined
            for fn in proj_items(0):
                fn()

            for w in range(NW):
                tq0 = w * W
                ntk = (w + 1) * (W // P)
                filler = []
                if w + 1 < NW:
                    filler += proj_items(w + 1)
                else:
                    # the last window has no projection filler; feed it all
                    # the deferred output projections instead
                    for wo_ in range(NW - 1):
                        filler += [(lambda m=m, w_=wo_: emit_o(m, w_))
                                   for m in range(C // P)]
                nsteps = CGP * ntk
                fidx = 0
                acc = 0.0
                per_step = len(filler) / nsteps
                for j in range(CGP):
                    # phase 1: scores + exp for all tk tiles (scalar-paced;
                    # filler keeps the PE busy), pt tiles persist
                    pts = []
                    for i in range(ntk):
                        vs0 = max(tq0, i * P)
                        n0 = tq0 + W - vs0
                        ps_s = psum_mm.tile([P, 2, W], f32, tag="ps")
                        pt = ptpool.tile([P, 2, W], bf16, tag="pt")
                        ts = slice(i * P, (i + 1) * P)
                        # K=64 per head at PE row offsets 0/64: the two
                        # matmuls run on separate array halves concurrently
                        mm(ps_s[:, 0, W - n0:], kT[0:HD, j, ts],
                           qT[0:HD, j, vs0:vs0 + n0], start=True, stop=True)
                        mm(ps_s[:, 1, W - n0:], kT[HD:P, j, ts],
                           qT[HD:P, j, vs0:vs0 + n0], start=True, stop=True)
                        nc.scalar.activation(pt[:, :, W - n0:],
                                             ps_s[:, :, W - n0:],
                                             AF.Exp, scale=escale)
                        if i * P >= tq0:  # diagonal tile: one dual-head mul
                            nc.vector.tensor_mul(
                                pt[:, :, W - n0:W - n0 + P],
                                pt[:, :, W - n0:W - n0 + P], maskD)
                        pts.append((pt, vs0, n0))
                        acc += per_step
                        while acc >= 1.0 and fidx < len(filler):
                            filler[fidx]()
                            fidx += 1
                            acc -= 1.0
                    # phase 2: per-head AV batches, double-buffered psum so
                    # head/group boundaries overlap with the norms
                    for a in range(2):
                        h = 2 * j + a
                        ps_av = psum_av.tile([VS, W], f32, tag="av")
                        for i in range(ntk):
                            pt, vs0, n0 = pts[i]
                            mm(ps_av[:, vs0 - tq0:],
                               vp[:, i, h * VS:(h + 1) * VS],
                               pt[:, a, W - n0:],
                               start=(i == 0), stop=(i == ntk - 1))
                        # normalize: reciprocal on the 1-row denominator,
                        # then broadcast the reciprocal
                        dn = dnpool.tile([1, W], f32, tag="dn")
                        nc.vector.tensor_copy(dn, ps_av[HD:VS, :])
                        nc.vector.reciprocal_approx_fast(out=dn, in_=dn)
                        rb = rbpool.tile([HD, W], f32, tag="rb")
                        nc.gpsimd.partition_broadcast(rb, dn)
                        nc.vector.tensor_mul(
                            yT[a * HD:(a + 1) * HD, j, tq0:tq0 + W],
                            ps_av[0:HD, :], rb)
                while fidx < len(filler):
                    filler[fidx]()
                    fidx += 1
            for m in range(C // P):
                emit_o(m, NW - 1)

    nc.compile()
    return nc


def get_nc():
    global _cached_nc
    if _cached_nc is None:
        _cached_nc = _build()
    return _cached_nc


def make_in_maps(x, Wq, bq, Wk, bk, Wv, bv, Wo, bo):
    x = np.asarray(x, np.float32)
    tri = np.triu(np.ones((P, P), np.float32))
    mask = np.concatenate([tri, tri], axis=1).astype(ml_dtypes.bfloat16)
    in_maps = []
    for c in range(8):
        b, g = c // 2, c % 2
        cs = slice(g * CG, (g + 1) * CG)
        in_maps.append({
            "xt": np.ascontiguousarray(x[b].T.astype(ml_dtypes.bfloat16)),
            "wq": np.ascontiguousarray(
                np.asarray(Wq, np.float32)[:, cs].astype(ml_dtypes.bfloat16)),
            "wk": np.ascontiguousarray(
                np.asarray(Wk, np.float32)[:, cs].astype(ml_dtypes.bfloat16)),
            "wv": np.ascontiguousarray(
                np.asarray(Wv, np.float32)[:, cs].astype(ml_dtypes.bfloat16)),
            "wo": np.ascontiguousarray(
                np.asarray(Wo, np.float32)[cs, :].astype(ml_dtypes.bfloat16)),
            "bq": np.ascontiguousarray(
                np.asarray(bq, np.float32)[cs].reshape(CGP, P).T),
            "bk": np.ascontiguousarray(
                np.asarray(bk, np.float32)[cs].reshape(CGP, P).T),
            "mask": np.ascontiguousarray(mask),
        })
    return in_maps


def combine(results, Wv, bv, Wo, bo):
    const = (np.asarray(bv, np.float32) @ np.asarray(Wo, np.float32)
             + np.asarray(bo, np.float32))
    out = np.empty((B, T, C), np.float32)
    for b in range(B):
        acc = (results[2 * b]["outp"].astype(np.float32)
               + results[2 * b + 1]["outp"].astype(np.float32))
        out[b] = acc.T + const[None, :]
    return out


def kernel(x, Wq, bq, Wk, bk, Wv, bv, Wo, bo):
    nc = get_nc()
    in_maps = make_in_maps(x, Wq, bq, Wk, bk, Wv, bv, Wo, bo)
    res = run_bass_kernel_spmd(nc, in_maps, core_ids=list(range(8)))
    return combine(res.results, Wv, bv, Wo, bo)
